# revision 1
# baseline (speedup 1.0000x reference)
"""Trainium2 Bass kernel for nn_BurgersSolver_75333726371954.

Burgers' equation explicit solver: interpolate u0 [64,512] to a 513-point
grid, run 5000 sequential periodic-stencil steps on [64,512], snapshot every
50th step at every 2nd spatial point -> [64,257,101].

Strategy (pure data parallel, batch sharded 8 rows/core across 8 cores):
  * Scaled state w = C1*u so the update is
        w' = (w+C2)*w_left - (w-C2)*w_right + (1-2*C2)*w
    = 4 standard DVE ops/step (2x scalar_tensor_tensor, tensor_sub, STT).
  * Layout [128 partitions = 8 batch x 16 spatial chunks of 32,
    free = 32 + 2H ghost columns]. Ghost zones allow H steps between
    partition-crossing halo exchanges; compute range tapers by 1/side/step.
  * Halo exchange via two TensorE permutation matmuls (bit-exact for fp32)
    into PSUM + one strided PSUM->SBUF copy, every H steps.
  * Snapshots: strided 1x-mode DVE tensor-add (copy) of the 16 even-spatial
    valid columns into an SBUF accumulation area; single DMA out at the end;
    host rescales by 1/C1 and assembles the [64,257,101] output.
  * A one-op writeback-margin spacer follows every in-place state update
    (DVE streaming reads at equal rate catch the previous op's writeback).
"""

import numpy as np

# ---- problem constants (hardcoded; must match the reference config) ----
MX = 513
MT = 5001
DX = 1.0 / (MX - 1)
DT = 1.0 / (MT - 1)
C1 = DT / (2.0 * DX)            # 0.0512
C2 = 0.005 * DT / DX ** 2       # 0.262144
LIN = float(1.0 - 2.0 * C2)

NSTEPS = MT - 1                 # 5000
SNAP_EVERY = 50
NSNAP = NSTEPS // SNAP_EVERY + 1  # 101

NCORES = 8
BPC = 8                         # batch rows per core
NCHUNK = 16                     # spatial chunks per batch row
CH = 32                         # chunk width (NCHUNK*CH == 512)
H = 20                         # ghost depth == steps between exchanges
W = CH + 2 * H                  # tile free width

_COMPILED = {}


def _build():
    import concourse.bass as bass
    import concourse.mybir as mybir

    F32 = mybir.dt.float32
    ALU = mybir.AluOpType

    nc = bass.Bass()
    x_in = nc.dram_tensor("x", [128, W], F32, kind="ExternalInput")
    pm_in = nc.dram_tensor("pm", [128, 256], F32, kind="ExternalInput")
    y_out = nc.dram_tensor("y", [128, NSNAP * 16], F32, kind="ExternalOutput")

    n_blocks = (NSTEPS + H - 1) // H
    assert NSTEPS % H == 0

    with (
        nc.semaphore("dma_sem") as dma_sem,
        nc.semaphore("x_sem") as x_sem,
        nc.semaphore("p_sem") as p_sem,
        nc.semaphore("v_sem") as v_sem,
        nc.sbuf_tensor("U", [128, W], F32) as U,
        nc.sbuf_tensor("T1", [128, W], F32) as T1,
        nc.sbuf_tensor("T2", [128, W], F32) as T2,
        nc.sbuf_tensor("S", [128, W], F32) as S,
        nc.sbuf_tensor("PM", [128, 256], F32) as PM,
        nc.sbuf_tensor("SN", [128, NSNAP * 16], F32) as SN,
        nc.sbuf_tensor("ZZ", [128, 1], F32) as ZZ,
        nc.psum_tensor("PS", [128, 2 * H], F32) as PS,
    ):
        # ghost-column destination view [128, 2, H]: cols [0,H) and [W-H, W)
        from concourse.ap import AP
        ubase = U[:]
        pstep = ubase.ap[0][0]
        ghost_dst = AP(ubase.tensor, 0, [[pstep, 128], [W - H, 2], [1, H]])
        psbase = PS[:]
        ps_step = psbase.ap[0][0]
        ps_src = AP(psbase.tensor, 0, [[ps_step, 128], [H, 2], [1, H]])

        with nc.Block() as block:
            @block.gpsimd
            def _(g):
                g.memset(ZZ[:], 0.0)
                g.dma_start(U[:], x_in[:]).then_inc(dma_sem, 16)
                g.dma_start(PM[:], pm_in[:]).then_inc(dma_sem, 16)

            zbc = ZZ[:].to_broadcast([128, 16])

            @block.vector
            def _(v):
                v.wait_ge(dma_sem, 32)
                # t=0 snapshot.  Snapshots use a 1x-mode tensor_tensor add
                # (a 2x-mode tensor_copy outpaces the previous op's SBUF
                # writeback and reads stale tail columns) plus a small spacer
                # op after the in-place state update.
                v.tensor_add(SN[:, 0:16], U[:, H:H + CH:2], zbc)
                def two_group(tile, off, width):
                    """[128, 2, width] view: cols [off, off+width) and
                    [off + W-H-2, ...) — the two step-1 edge ranges."""
                    base = tile[:]
                    return AP(base.tensor, off,
                              [[base.ap[0][0], 128], [W - H - 2, 2], [1, width]])

                step = 0
                snap = 1
                pending_snap = False
                for blk in range(n_blocks):
                    if blk > 0:
                        # --- step 1, split around the ghost wait ----------
                        # Interior piece reads only core columns (valid before
                        # the exchange lands) and writes scratch, so it hides
                        # under the PE round trip.  Edge piece runs after the
                        # ghost copy.  The in-place state write (un) stays
                        # whole.
                        loI, hiI = H + 1, W - H - 1
                        v.scalar_tensor_tensor(T1[:, loI:hiI], U[:, loI:hiI],
                                               C2, U[:, loI - 1:hiI - 1],
                                               ALU.add, ALU.mult)
                        v.scalar_tensor_tensor(T2[:, loI:hiI], U[:, loI:hiI],
                                               C2, U[:, loI + 1:hiI + 1],
                                               ALU.subtract, ALU.mult)
                        if pending_snap:
                            v.tensor_add(SN[:, snap * 16:snap * 16 + 16],
                                         U[:, H:H + CH:2], zbc)
                            snap += 1
                            pending_snap = False
                        v.tensor_sub(S[:, loI:hiI], T1[:, loI:hiI],
                                     T2[:, loI:hiI])
                        v.wait_ge(p_sem, blk)
                        v.tensor_copy(ghost_dst, ps_src)
                        v.scalar_tensor_tensor(two_group(T1, 1, H),
                                               two_group(U, 1, H), C2,
                                               two_group(U, 0, H),
                                               ALU.add, ALU.mult)
                        v.scalar_tensor_tensor(two_group(T2, 1, H),
                                               two_group(U, 1, H), C2,
                                               two_group(U, 2, H),
                                               ALU.subtract, ALU.mult)
                        v.tensor_sub(two_group(S, 1, H), two_group(T1, 1, H),
                                     two_group(T2, 1, H))
                        v.scalar_tensor_tensor(U[:, 1:W - 1], U[:, 1:W - 1],
                                               LIN, S[:, 1:W - 1],
                                               ALU.mult, ALU.add)
                        v.tensor_sub(S[:, 0:2], T1[:, 0:2], T2[:, 0:2])
                        step += 1
                        if step % SNAP_EVERY == 0:
                            pending_snap = True
                        s_start = 2
                    else:
                        s_start = 1
                    for s in range(s_start, H + 1):
                        lo, hi = s, W - s
                        c = U[:, lo:hi]
                        l = U[:, lo - 1:hi - 1]
                        r = U[:, lo + 1:hi + 1]
                        v.scalar_tensor_tensor(T1[:, lo:hi], c, C2, l,
                                               ALU.add, ALU.mult)
                        v.scalar_tensor_tensor(T2[:, lo:hi], c, C2, r,
                                               ALU.subtract, ALU.mult)
                        if pending_snap:
                            # snapshot of the PREVIOUS step's state: U's core
                            # columns are untouched since then, and the two
                            # STT ops above give the writeback margin.
                            v.tensor_add(SN[:, snap * 16:snap * 16 + 16],
                                         U[:, H:H + CH:2], zbc)
                            snap += 1
                            pending_snap = False
                        v.tensor_sub(S[:, lo:hi], T1[:, lo:hi], T2[:, lo:hi])
                        un = v.scalar_tensor_tensor(c, c, LIN, S[:, lo:hi],
                                                    ALU.mult, ALU.add)
                        # writeback-margin spacer: the next op reads U at the
                        # same streaming rate the in-place update wrote it;
                        # without a gap it can read stale columns.  At block
                        # ends this hides under the exchange stall anyway.
                        v.tensor_sub(S[:, 0:2], T1[:, 0:2], T2[:, 0:2])
                        step += 1
                        if blk < n_blocks - 1 and s == H:
                            un.then_inc(x_sem, 1)
                        if step % SNAP_EVERY == 0:
                            pending_snap = True
                # final snapshot (step == NSTEPS): two spacer ops, then read
                v.tensor_sub(S[:, 0:4], T1[:, 0:4], T2[:, 0:4])
                v.tensor_sub(S[:, 4:8], T1[:, 4:8], T2[:, 4:8])
                v.tensor_add(SN[:, snap * 16:snap * 16 + 16],
                             U[:, H:H + CH:2], zbc).then_inc(v_sem, 1)

            @block.tensor
            def _(t):
                for k in range(1, n_blocks):
                    t.wait_ge(x_sem, k)
                    t.matmul(PS[:, 0:H], PM[:, 0:128], U[:, CH:CH + H],
                             start=True, stop=True)
                    t.matmul(PS[:, H:2 * H], PM[:, 128:256], U[:, H:2 * H],
                             start=True, stop=True).then_inc(p_sem, 1)

            @block.gpsimd
            def _(g):
                g.wait_ge(v_sem, 1)
                g.dma_start(y_out[:], SN[:]).then_inc(dma_sem, 16)
                g.wait_ge(dma_sem, 48)

    return nc


def _perm_inputs():
    """[128, 256] fp32: lhsT_L | lhsT_R permutation matrices.

    out[m,:] = sum_k lhsT[k,m] * rhs[k,:]  ->  lhsT[src(m), m] = 1.
    Left ghosts come from chunk c-1, right ghosts from chunk c+1 (mod 16,
    within the same batch group of 16 partitions).
    """
    pm = np.zeros((128, 256), dtype=np.float32)
    for m in range(128):
        b, c = divmod(m, NCHUNK)
        src_l = b * NCHUNK + (c - 1) % NCHUNK
        src_r = b * NCHUNK + (c + 1) % NCHUNK
        pm[src_l, m] = 1.0
        pm[src_r, 128 + m] = 1.0
    return pm


def _interp_init(u0):
    """Replicate the reference's 1D border-padded linear interp, f32."""
    u0 = np.asarray(u0, dtype=np.float32)
    n_in = u0.shape[1]
    X = np.linspace(0.0, 1.0, MX, dtype=np.float32)
    pts = X * np.float32(2.0) - np.float32(1.0)
    idx = (pts + np.float32(1.0)) * np.float32(0.5) * np.float32(n_in - 1)
    idx = np.clip(idx, 0.0, np.float32(n_in - 1))
    i0 = np.floor(idx).astype(np.int32)
    i0 = np.clip(i0, 0, n_in - 2)
    frac = (idx - i0.astype(np.float32)).astype(np.float32)
    u0f = u0[:, i0] * (np.float32(1.0) - frac) + u0[:, i0 + 1] * frac
    return u0f[:, :-1].astype(np.float32)   # [B, 512]


def kernel(u0):
    from concourse.bass_utils import run_bass_kernel_spmd

    u0 = np.asarray(u0, dtype=np.float32)
    B = u0.shape[0]
    assert B == NCORES * BPC and u0.shape[1] == 512

    u_init = _interp_init(u0)                       # [64, 512]
    w0 = (np.float32(C1) * u_init).astype(np.float32)

    # build per-core input tiles [128, W] with pre-filled ghosts
    cc, xx = np.meshgrid(np.arange(NCHUNK), np.arange(W), indexing="ij")
    src = (cc * CH + xx - H) % 512                  # [16, W]
    pm = _perm_inputs()
    in_maps = []
    for core in range(NCORES):
        wrows = w0[core * BPC:(core + 1) * BPC]     # [8, 512]
        tile = wrows[:, src].astype(np.float32)     # [8, 16, W]
        in_maps.append({"x": tile.reshape(128, W), "pm": pm})

    if "nc" not in _COMPILED:
        _COMPILED["nc"] = _build()
    nc = _COMPILED["nc"]

    res = run_bass_kernel_spmd(nc, in_maps, core_ids=list(range(NCORES)))

    out = np.empty((B, 257, NSNAP), dtype=np.float32)
    inv_c1 = np.float32(1.0 / C1)
    for core in range(NCORES):
        y = res.results[core]["y"]                  # [128, NSNAP*16]
        y = y.reshape(BPC, NCHUNK, NSNAP, 16)       # [b, chunk, t, k]
        u = y * inv_c1
        # spatial index nx = chunk*16 + k  (covers 0..255)
        out[core * BPC:(core + 1) * BPC, 0:256, :] = (
            u.transpose(0, 1, 3, 2).reshape(BPC, 256, NSNAP))
    out[:, 256, :] = out[:, 0, :]
    return out



# revision 5
# speedup vs baseline: 4.1109x; 4.1109x over previous
"""Trainium2 Bass kernel for nn_BurgersSolver_75333726371954.

Burgers' equation explicit solver: interpolate u0 [64,512] to a 513-point
grid, run 5000 sequential periodic-stencil steps on [64,512], snapshot every
50th step at every 2nd spatial point -> [64,257,101].

Strategy (pure data parallel, batch sharded 8 rows/core across 8 cores):
  * Affine-scaled state v = C1*u + C2 makes the update constant-free:
        vn[x] = (v[x-1] - v[x+1] + LIN)*v[x] + 2*C2*v[x+1],  LIN = 1-2*C2.
  * A hand-written custom DVE uop (BURGERS_STEP_ANT) computes one ENTIRE
    time step in ONE vector instruction: Src0 streams v[x+1], Src1 streams
    v[x-1], and v[x] is recovered on-chip from block 0's ALU-flop history
    (CURR_ALU_OUT temporal read) -- a 6-stage datapath program.
  * Layout [128 partitions = 8 batch rows x 16 spatial chunks of 32,
    free = 2 scratch + H ghosts + 32 core + H ghosts]. Every step is the
    same full-width instruction; the first output element of each
    instruction is garbage and lands exactly on the column the ghost taper
    abandons. Long constant streams keep the next instruction's reads
    clear of the previous one's SBUF writeback.
  * Ghost refresh every H steps with two STREAM_SHUFFLE copies (32-lane
    partition permutation, same pattern in all four quadrants) -- no PE,
    no cross-engine semaphores.
  * Snapshots: strided copy of the 16 even core columns into an SBUF
    accumulator; one DMA out at the end; host decodes u = (v - C2)/C1.
"""

import numpy as np

# ---- problem constants (hardcoded; must match the reference config) ----
MX = 513
MT = 5001
DX = 1.0 / (MX - 1)
DT = 1.0 / (MT - 1)
C1 = DT / (2.0 * DX)            # 0.0512
C2 = 0.005 * DT / DX ** 2       # 0.262144
LIN = float(1.0 - 2.0 * C2)
TWO_C2 = float(2.0 * C2)

NSTEPS = MT - 1                 # 5000
SNAP_EVERY = 50
NSNAP = NSTEPS // SNAP_EVERY + 1  # 101

NCORES = 8
BPC = 8                         # batch rows per core
NCHUNK = 16                     # spatial chunks per batch row
CH = 32                         # chunk width (NCHUNK*CH == 512)
H = 10                          # ghost depth == steps between exchanges
W = CH + 2 * H                  # state width per partition
T = W + 2                       # + 2 scratch columns (0 and T-1)
CORE0 = 1 + H                   # first core column in the tile

_COMPILED = {}

# ---------------------------------------------------------------------------
# custom DVE op: one Burgers step per instruction
# ---------------------------------------------------------------------------

_DVE_OP = {}


def _register_dve_op():
    if "op" in _DVE_OP:
        return _DVE_OP["op"]
    import concourse.dve_ops as dve_ops
    from concourse.dve_spec import Spec, Src0, Src1
    from concourse.dve_uop import (
        ENABLE,
        AluInp,
        AluOp,
        DelayInp,
        DveOpSpec,
        InpSel,
        OutPath,
        OutSel,
        Trigger,
        UopConfig,
        UopDpConfig,
    )

    name = "BURGERS_STEP_ANT"

    def build_uop():
        u = UopConfig()
        u.enable_input(InpSel.SRC_0, 1)    # PREV_DELAY_0 = r = v[x+1]
        u.enable_input(InpSel.SRC_1, 2)    # PREV_DELAY_1 = l = v[x-1]
        u.enable_input(InpSel.CONST_0, 3)  # PREV_DELAY_2 = LIN
        u.enable_input(InpSel.CONST_1, 4)  # PREV_DELAY_3 = 2*C2
        u.require_inp0 = ENABLE
        u.require_inp1 = ENABLE
        u.trigger = (Trigger.SRC_TENSOR_DONE, Trigger.NONE, Trigger.NONE)
        u.enable_output(OutSel.ALU_OUT, OutPath.WR0_LO)
        dp = u.datapath_config
        # blk0: flop0 <- r; lane4 <- previous element's flop0 (= c = v[x])
        dp[0] = (
            UopDpConfig()
            .enable_alu(AluOp.BYPASS, AluInp.PREV_DELAY_0, AluInp.PREV_DELAY_0)
            .pass_through_delay(0, 1, 2, 3)
            .enable_delay_from_src(DelayInp.CURR_ALU_OUT, 4)
        )
        # blk1: d = l - r
        dp[1] = (
            UopDpConfig()
            .enable_alu(AluOp.SUBTRACT, AluInp.PREV_DELAY_1, AluInp.PREV_ALU_OUT)
            .pass_through_delay(0, 2, 3, 4)
        )
        # blk2: dl = d + LIN
        dp[2] = (
            UopDpConfig()
            .enable_alu(AluOp.ADD, AluInp.PREV_ALU_OUT, AluInp.PREV_DELAY_2)
            .pass_through_delay(0, 3, 4)
        )
        # blk3: t = dl * c
        dp[3] = (
            UopDpConfig()
            .enable_alu(AluOp.MULTIPLY, AluInp.PREV_ALU_OUT, AluInp.PREV_DELAY_4)
            .pass_through_delay(0, 3)
        )
        # blk4: q = r * 2C2 ; lane0 <- t
        dp[4] = (
            UopDpConfig()
            .enable_alu(AluOp.MULTIPLY, AluInp.PREV_DELAY_0, AluInp.PREV_DELAY_3)
            .enable_delay_from_src(DelayInp.PREV_ALU_OUT, 0)
        )
        # blk5: vn = q + t
        dp[5] = UopDpConfig().enable_alu(
            AluOp.ADD, AluInp.PREV_ALU_OUT, AluInp.PREV_DELAY_0
        )
        dp[6] = UopDpConfig().pass_through_alu()
        dp[7] = UopDpConfig().pass_through_alu()
        return u

    def reference(in0, in1, c0, c1, c2):
        in0 = np.asarray(in0, np.float32)
        in1 = np.asarray(in1, np.float32)
        c0 = np.float32(np.asarray(c0).reshape(-1)[0] if np.ndim(c0) else c0)
        c1 = np.float32(np.asarray(c1).reshape(-1)[0] if np.ndim(c1) else c1)
        P = in0.shape[0]
        r = in0.reshape(P, -1)
        l = in1.reshape(P, -1)
        out = np.zeros_like(r)
        out[:, 1:] = (l[:, 1:] - r[:, 1:] + c0) * r[:, :-1] + c1 * r[:, 1:]
        return out.reshape(in0.shape)

    class HandDveOp(dve_ops.DveOp):
        def compile(self, ver):
            key = (self.name, ver)
            cached = dve_ops._COMPILE_CACHE.get(key)
            if cached is not None:
                return cached
            result = DveOpSpec(
                name=self.name,
                opcode=dve_ops.get_dve_sub_opcode(self.name),
                uops=[build_uop()],
                rd1_en=True,
            )
            result.validate(ver)
            dve_ops._COMPILE_CACHE[key] = result
            return result

    for op in dve_ops.OPS:
        if op.name == name:
            _DVE_OP["op"] = op
            return op
    op = HandDveOp(
        name, Spec(body=Src0 - Src1, reference=reference), subdim=False, uops_sha={}
    )
    dve_ops.OPS.append(op)
    dve_ops._SUB_OPCODE_FOR_NAME[name] = 1 + max(
        dve_ops._SUB_OPCODE_FOR_NAME.values()
    )
    assert dve_ops._SUB_OPCODE_FOR_NAME[name] < 0x20
    dve_ops.CUSTOM_DVE_SPECS[name] = op.spec
    _DVE_OP["op"] = op
    return op


# ---------------------------------------------------------------------------
# shuffle masks: 32-lane permutation, lane l = (row&1)*16 + chunk
# ---------------------------------------------------------------------------

def _masks():
    maskL = [(l & 16) | ((l - 1) & 15) for l in range(32)]  # ghost <- chunk c-1
    maskR = [(l & 16) | ((l + 1) & 15) for l in range(32)]  # ghost <- chunk c+1
    return maskL, maskR


# ---------------------------------------------------------------------------
# kernel build
# ---------------------------------------------------------------------------

def _build():
    import concourse.bass as bass
    import concourse.mybir as mybir

    op = _register_dve_op()
    maskL, maskR = _masks()

    F32 = mybir.dt.float32
    nc = bass.Bass()
    x_in = nc.dram_tensor("x", [128, T], F32, kind="ExternalInput")
    y_out = nc.dram_tensor("y", [128, NSNAP * 16], F32, kind="ExternalOutput")

    assert NSTEPS % H == 0 and SNAP_EVERY % H == 0

    with (
        nc.semaphore("dma_sem") as dma_sem,
        nc.semaphore("v_sem") as v_sem,
        nc.sbuf_tensor("U", [128, T], F32) as U,
        nc.sbuf_tensor("SN", [128, NSNAP * 16], F32) as SN,
        nc.sbuf_tensor("SP", [128, 80], F32) as SP,
    ):
        with nc.Block() as block:
            @block.gpsimd
            def _(g):
                g.memset(SP[:], 0.0)
                g.dma_start(U[:], x_in[:]).then_inc(dma_sem, 16)

            @block.vector
            def _(v):
                v.wait_ge(dma_sem, 16)
                core_even = U[:, CORE0:CORE0 + CH:2]
                v.tensor_copy(SN[:, 0:16], core_even)   # t=0 snapshot
                snap = 1
                for t in range(1, NSTEPS + 1):
                    v._custom_dve(
                        op,
                        out=U[:, 1:T - 1],
                        in0=U[:, 2:T],
                        in1=U[:, 0:T - 2],
                        s0=LIN,
                        s1=TWO_C2,
                    )
                    if t % H == 0 and t < NSTEPS:
                        # wide writeback-margin spacer: shufL reads the step's
                        # late-written core-tail columns.  Order L-then-R so
                        # shufR separates shufL's ghost writes from the next
                        # step's early left-ghost reads; the right ghosts are
                        # only read ~40 elements into the next step's stream.
                        v.tensor_copy(SP[:, 0:36], SP[:, 40:76])
                        v.stream_shuffle(
                            U[:, 1:1 + H],
                            U[:, 1 + CH:1 + CH + H], maskL)
                        v.stream_shuffle(
                            U[:, 1 + H + CH:1 + H + CH + H],
                            U[:, 1 + H:1 + 2 * H], maskR)
                    if t % SNAP_EVERY == 0:
                        if t % H != 0 or t == NSTEPS:
                            v.tensor_copy(SP[:, 0:4], SP[:, 4:8])
                            v.tensor_copy(SP[:, 4:8], SP[:, 0:4])
                        s = v.tensor_copy(SN[:, snap * 16:snap * 16 + 16],
                                          core_even)
                        snap += 1
                        if t == NSTEPS:
                            s.then_inc(v_sem, 1)

            @block.gpsimd
            def _(g):
                g.wait_ge(v_sem, 1)
                g.dma_start(y_out[:], SN[:]).then_inc(dma_sem, 16)
                g.wait_ge(dma_sem, 32)

    mybir.codegen_inst_isa_subclasses(nc)
    return nc


# ---------------------------------------------------------------------------
# host side
# ---------------------------------------------------------------------------

def _interp_init(u0):
    """Replicate the reference's 1D border-padded linear interp, f32."""
    u0 = np.asarray(u0, dtype=np.float32)
    n_in = u0.shape[1]
    X = np.linspace(0.0, 1.0, MX, dtype=np.float32)
    pts = X * np.float32(2.0) - np.float32(1.0)
    idx = (pts + np.float32(1.0)) * np.float32(0.5) * np.float32(n_in - 1)
    idx = np.clip(idx, 0.0, np.float32(n_in - 1))
    i0 = np.floor(idx).astype(np.int32)
    i0 = np.clip(i0, 0, n_in - 2)
    frac = (idx - i0.astype(np.float32)).astype(np.float32)
    u0f = u0[:, i0] * (np.float32(1.0) - frac) + u0[:, i0 + 1] * frac
    return u0f[:, :-1].astype(np.float32)   # [B, 512]


def _tiles(u0):
    """Per-core [128, T] state tiles of v = C1*u + C2 with ghosts filled."""
    u_init = _interp_init(u0)                       # [64, 512]
    v0 = (np.float32(C1) * u_init + np.float32(C2)).astype(np.float32)
    cc, jj = np.meshgrid(np.arange(NCHUNK), np.arange(T), indexing="ij")
    src = (cc * CH + jj - H - 1) % 512              # [16, T]
    tiles = []
    for core in range(NCORES):
        rows = v0[core * BPC:(core + 1) * BPC]      # [8, 512]
        tiles.append(rows[:, src].astype(np.float32).reshape(128, T))
    return tiles


def kernel(u0):
    from concourse.bass_utils import run_bass_kernel_spmd

    u0 = np.asarray(u0, dtype=np.float32)
    B = u0.shape[0]
    assert B == NCORES * BPC and u0.shape[1] == 512

    in_maps = [{"x": t} for t in _tiles(u0)]

    if "nc" not in _COMPILED:
        _COMPILED["nc"] = _build()
    nc = _COMPILED["nc"]

    res = run_bass_kernel_spmd(nc, in_maps, core_ids=list(range(NCORES)))

    out = np.empty((B, 257, NSNAP), dtype=np.float32)
    inv_c1 = np.float32(1.0 / C1)
    c2 = np.float32(C2)
    for core in range(NCORES):
        y = res.results[core]["y"]                  # [128, NSNAP*16]
        y = y.reshape(BPC, NCHUNK, NSNAP, 16)       # [b, chunk, t, k]
        u = (y - c2) * inv_c1
        # spatial index nx = chunk*16 + k  (covers 0..255)
        out[core * BPC:(core + 1) * BPC, 0:256, :] = (
            u.transpose(0, 1, 3, 2).reshape(BPC, 256, NSNAP))
    out[:, 256, :] = out[:, 0, :]
    return out
